# revision 11
# baseline (speedup 1.0000x reference)
"""nn_FM_49701361549558 — FM embedding lookup on 8 TRN2 NeuronCores.

Sharding: data-parallel over the batch (16384 -> 8 x 2048) combined with
row-sharding of the embedding tables (per the sharding hint): each core's
HBM holds exactly the table rows its batch shard references — user/item
rows deduplicated to <=2048 each, the two small meta tables shipped whole.
That keeps every per-core row index under 2^15, which lets the device use
the production `dma_gather` ucode (per-index gather, int16 indices; the
generic `indirect_dma_start` path only supports one dynamic offset per
partition on real HW, so it cannot express this access pattern).

Per-core fused table [25096, 128] bf16; row = [emb (64) | lin_hi | lin_lo |
pad to 256 B] (256 B rows are a dma_gather requirement; lin = lin_hi +
lin_lo double-bf16 keeps the O(1) linear term at ~f32 precision, while the
O(1/64) embedding factors tolerate bf16 easily — measured end-to-end
absmax error vs f64 is ~2e-5).

Device kernel (raw bass, identical SPMD program on all 8 cores; batch item
b = p*16 + t lives at partition p, slot t — output lands flat-contiguous,
no transposes anywhere):
  * 3 chunked dma_gathers (5/5/6 t-slots, 512*TC indices each) on GPSIMD,
    pipelined against compute with explicit semaphores,
  * DVE computes the FM pairwise term via the 3-product identity
      U*I + M0*M1 + (U+I)*(M0+M1) == sum over all 6 field pairs v_i*v_j
                                  == 0.5*(s^2 - sum_f v^2)
    (7 elementwise bf16 ops, 2x DVE mode), reduced over the 64 factors,
  * the linear term is one reduce over the 4 fields' (hi, lo) columns,
  * ACT applies sigmoid; per-chunk [128, TC] f32 results are DMA'd out on
    the HWDGE (sync) engine.
"""

import contextlib

import numpy as np
import ml_dtypes

import concourse.mybir as mybir
from concourse import bacc
from concourse.bass_utils import run_bass_kernel_spmd

P = 128
B = 16384
N_CORES = 8
BL = B // N_CORES          # 2048 per core
T = BL // P                # 16 t-slots
F = 64
NF = 4                     # fields: user, item, meta0, meta1
N_USERS = 1_000_000
N_ITEMS = 100_000
N_M0 = 1_000
N_M1 = 20_000

f32 = mybir.dt.float32
i16 = mybir.dt.int16
bf16 = mybir.dt.bfloat16

# t-slots per chunk; 2 t-slots = 1024 gather indices = the max one
# dma_gather can carry (the SWDGE descriptor ring holds 1024 descriptors;
# num_idxs=2048 faults the exec unit, 1024 is verified good on HW).
CHUNKS = (2, 2, 2, 2, 2, 2, 2, 2)
ELEM = 128                 # bf16 elements per table row (= 256 B, required)
NLIN = 2                   # lin_hi, lin_lo columns
SHARD_N = (BL, BL, N_M0, N_M1)               # user, item, meta0, meta1
BASES = (0, BL, 2 * BL, 2 * BL + N_M0)
N_ROWS = 2 * BL + N_M0 + N_M1                # 25096 (< 2^15)
N_IDX = NF * BL                              # 8192 gathered rows per core


def build_nc():
    nc = bacc.Bacc(None, target_bir_lowering=False)
    idx_d = nc.declare_dram_parameter("idx", [P, N_IDX // 16], i16, isOutput=False)
    tab_d = nc.declare_dram_parameter("table", [N_ROWS, ELEM], bf16, isOutput=False)
    out_d = nc.declare_dram_parameter("out", [P, T], f32, isOutput=True)

    C = len(CHUNKS)
    with contextlib.ExitStack() as ctx:
        idx_sb = ctx.enter_context(nc.sbuf_tensor("idx_sb", [P, N_IDX // 16], i16))
        gs = [
            ctx.enter_context(nc.sbuf_tensor(f"g{k}", [P, NF * TC, ELEM], bf16))
            for k, TC in enumerate(CHUNKS)
        ]
        tmps = [
            [
                ctx.enter_context(nc.sbuf_tensor(f"tmp{k}_{j}", [P, TC, F], bf16))
                for j in range(7)
            ]
            for k, TC in enumerate(CHUNKS)
        ]
        pws = [ctx.enter_context(nc.sbuf_tensor(f"pw{k}", [P, TC], bf16))
               for k, TC in enumerate(CHUNKS)]
        ls = [ctx.enter_context(nc.sbuf_tensor(f"l{k}", [P, TC], f32))
              for k, TC in enumerate(CHUNKS)]
        zs = [ctx.enter_context(nc.sbuf_tensor(f"z{k}", [P, TC], f32))
              for k, TC in enumerate(CHUNKS)]
        sig = ctx.enter_context(nc.sbuf_tensor("sig", [P, T], f32))
        isem = ctx.enter_context(nc.semaphore("isem"))   # idx DMA, +16
        gsems = [ctx.enter_context(nc.semaphore(f"gsem{k}"))
                 for k in range(C)]                          # per-chunk gather DMA
        vsem = ctx.enter_context(nc.semaphore("vsem"))   # DVE chunk done, +1
        ssem = ctx.enter_context(nc.semaphore("ssem"))   # ACT chunk done, +1
        osem = ctx.enter_context(nc.semaphore("osem"))   # out DMAs, +16 each
        block = ctx.enter_context(nc.Block())

        @block.gpsimd
        def _(gpsimd):
            gpsimd.dma_start(out=idx_sb[:], in_=idx_d[:]).then_inc(isem, 16)
            gpsimd.wait_ge(isem, 16)
            c0 = 0
            for k, TC in enumerate(CHUNKS):
                n_k = 128 * NF * TC
                gpsimd.dma_gather(
                    out_ap=gs[k][:],
                    in_ap=tab_d[:],
                    idxs_ap=idx_sb[:, c0:c0 + n_k // 16],
                    num_idxs=n_k,
                    num_idxs_reg=n_k,
                    elem_size=ELEM,
                ).then_inc(gsems[k], 16)
                c0 += n_k // 16

        @block.vector
        def _(vector):
            t0 = 0
            for k, TC in enumerate(CHUNKS):
                vector.wait_ge(gsems[k], 16)
                g4 = gs[k][:].rearrange("p (f t) e -> p f t e", f=NF)
                U = g4[:, 0, :, 0:F]
                I = g4[:, 1, :, 0:F]
                M0 = g4[:, 2, :, 0:F]
                M1 = g4[:, 3, :, 0:F]
                a, b, x, y, w, t2, d2 = tmps[k]
                vector.tensor_mul(out=x[:], in0=U, in1=I)
                vector.tensor_mul(out=y[:], in0=M0, in1=M1)
                vector.tensor_add(out=a[:], in0=U, in1=I)
                vector.tensor_add(out=b[:], in0=M0, in1=M1)
                vector.drain()          # DVE same-engine RAW needs a drain
                vector.tensor_mul(out=w[:], in0=a[:], in1=b[:])
                vector.tensor_add(out=t2[:], in0=x[:], in1=y[:])
                vector.drain()
                vector.tensor_add(out=d2[:], in0=t2[:], in1=w[:])
                vector.drain()
                with nc.allow_low_precision("FM pairwise term is O(0.05)"):
                    vector.tensor_reduce(
                        out=pws[k][:], in_=d2[:],
                        axis=mybir.AxisListType.X, op=mybir.AluOpType.add,
                    )
                lv = g4[:, :, :, F:F + NLIN].rearrange("p f t e -> p t f e")
                vector.tensor_reduce(
                    out=ls[k][:], in_=lv,
                    axis=mybir.AxisListType.XY, op=mybir.AluOpType.add,
                )
                vector.drain()
                vector.tensor_add(
                    out=zs[k][:], in0=pws[k][:], in1=ls[k][:]
                ).then_inc(vsem, 1)
                t0 += TC

        @block.scalar
        def _(scalar):
            t0 = 0
            for k, TC in enumerate(CHUNKS):
                scalar.wait_ge(vsem, k + 1)
                scalar.activation(
                    out=sig[:, t0:t0 + TC], in_=zs[k][:],
                    func=mybir.ActivationFunctionType.Sigmoid,
                ).then_inc(ssem, 1)
                t0 += TC

        @block.sync
        def _(sync):
            t0 = 0
            for k, TC in enumerate(CHUNKS):
                sync.wait_ge(ssem, k + 1)
                sync.dma_start(
                    out=out_d[:, t0:t0 + TC], in_=sig[:, t0:t0 + TC]
                ).then_inc(osem, 16)
                t0 += TC
            sync.wait_ge(osem, 16 * C)

    nc.finalize()
    return nc


def _fill_rows(block, emb, lin):
    """block[:, :] = [emb | lin_hi | lin_lo | 0-pad] in bf16."""
    block[:, :F] = emb                          # f32 -> bf16 cast
    hi = lin.astype(ml_dtypes.bfloat16)
    block[:, F] = hi
    block[:, F + 1] = lin - hi.astype(np.float32)
    block[:, F + NLIN:] = 0


def host_prepare(inputs):
    """Row-shard the tables per core and build device index tensors."""
    user_emb = np.asarray(inputs["user_emb"], np.float32)
    item_emb = np.asarray(inputs["item_emb"], np.float32)
    user_lin = np.asarray(inputs["user_lin"], np.float32).reshape(-1)
    item_lin = np.asarray(inputs["item_lin"], np.float32).reshape(-1)

    uids = np.asarray(inputs["user_ids"]).astype(np.int64)
    iids = np.asarray(inputs["item_ids"]).astype(np.int64)
    meta = np.asarray(inputs["metadata_ids"]).astype(np.int64)

    # meta blocks are shared by all cores
    meta_block = np.zeros((N_M0 + N_M1, ELEM), ml_dtypes.bfloat16)
    _fill_rows(meta_block[:N_M0], np.asarray(inputs["meta_emb0"], np.float32),
               np.asarray(inputs["meta_lin0"], np.float32).reshape(-1))
    _fill_rows(meta_block[N_M0:], np.asarray(inputs["meta_emb1"], np.float32),
               np.asarray(inputs["meta_lin1"], np.float32).reshape(-1))

    per_core_idx, per_core_tab = [], []
    for c in range(N_CORES):
        sl = slice(c * BL, (c + 1) * BL)
        u_uniq, u_inv = np.unique(uids[sl], return_inverse=True)
        i_uniq, i_inv = np.unique(iids[sl], return_inverse=True)

        tab = np.zeros((N_ROWS, ELEM), ml_dtypes.bfloat16)
        _fill_rows(tab[:len(u_uniq)], user_emb[u_uniq], user_lin[u_uniq])
        _fill_rows(tab[BL:BL + len(i_uniq)], item_emb[i_uniq], item_lin[i_uniq])
        tab[2 * BL:] = meta_block

        # local row index per field, [NF, P, T] (batch item b = p*16 + t)
        loc = np.empty((NF, P, T), np.int16)
        loc[0] = (u_inv + BASES[0]).reshape(P, T)
        loc[1] = (i_inv + BASES[1]).reshape(P, T)
        loc[2] = (meta[sl, 0] + BASES[2]).reshape(P, T)
        loc[3] = (meta[sl, 1] + BASES[3]).reshape(P, T)

        # unwrapped gather order: index j -> out[p = j%128, col = j//128];
        # col = f*TC + tt within a chunk
        blocks = []
        t0 = 0
        for TC in CHUNKS:
            u_k = np.ascontiguousarray(
                loc[:, :, t0:t0 + TC].transpose(0, 2, 1)   # [f, tt, p]
            ).reshape(-1)                                   # j = (f*TC+tt)*128+p
            blocks.append(u_k.reshape(-1, 16).T)            # [16, n_k/16]
            t0 += TC
        idx16 = np.concatenate(blocks, axis=1)              # [16, N_IDX/16]
        per_core_idx.append(np.tile(idx16, (P // 16, 1)))   # replicate to 128
        per_core_tab.append(tab)
    return per_core_idx, per_core_tab


_NC_CACHE = None


def _get_nc():
    global _NC_CACHE
    if _NC_CACHE is None:
        _NC_CACHE = build_nc()
    return _NC_CACHE


def kernel(**inputs) -> np.ndarray:
    nc = _get_nc()
    per_core_idx, per_core_tab = host_prepare(inputs)
    in_maps = [
        {"idx": per_core_idx[c], "table": per_core_tab[c]}
        for c in range(N_CORES)
    ]
    res = run_bass_kernel_spmd(nc, in_maps, list(range(N_CORES)))
    return np.concatenate(
        [res.results[c]["out"].reshape(-1) for c in range(N_CORES)]
    ).astype(np.float32)


# revision 12
# speedup vs baseline: 1.0186x; 1.0186x over previous
"""nn_FM_49701361549558 — FM embedding lookup on 8 TRN2 NeuronCores.

Sharding: data-parallel over the batch (16384 -> 8 x 2048) combined with
row-sharding of the embedding tables (per the sharding hint): each core's
HBM holds exactly the table rows its batch shard references — user/item
rows deduplicated to <=2048 each, the two small meta tables shipped whole.
That keeps every per-core row index under 2^15, which lets the device use
the production `dma_gather` ucode (per-index gather, int16 indices; the
generic `indirect_dma_start` path only supports one dynamic offset per
partition on real HW, so it cannot express this access pattern).

Per-core fused table [25096, 128] bf16; row = [emb (64) | lin_hi | lin_lo |
pad to 256 B] (256 B rows are a dma_gather requirement; lin = lin_hi +
lin_lo double-bf16 keeps the O(1) linear term at ~f32 precision, while the
O(1/64) embedding factors tolerate bf16 easily — measured end-to-end
absmax error vs f64 is ~2e-5).

Device kernel (raw bass, identical SPMD program on all 8 cores; batch item
b = p*16 + t lives at partition p, slot t — output lands flat-contiguous,
no transposes anywhere):
  * 3 chunked dma_gathers (5/5/6 t-slots, 512*TC indices each) on GPSIMD,
    pipelined against compute with explicit semaphores,
  * DVE computes the FM pairwise term via the 3-product identity
      U*I + M0*M1 + (U+I)*(M0+M1) == sum over all 6 field pairs v_i*v_j
                                  == 0.5*(s^2 - sum_f v^2)
    (7 elementwise bf16 ops, 2x DVE mode), reduced over the 64 factors,
  * the linear term is one reduce over the 4 fields' (hi, lo) columns,
  * ACT applies sigmoid; per-chunk [128, TC] f32 results are DMA'd out on
    the HWDGE (sync) engine.
"""

import contextlib

import numpy as np
import ml_dtypes

import concourse.mybir as mybir
from concourse import bacc
from concourse.bass_utils import run_bass_kernel_spmd

P = 128
B = 16384
N_CORES = 8
BL = B // N_CORES          # 2048 per core
T = BL // P                # 16 t-slots
F = 64
NF = 4                     # fields: user, item, meta0, meta1
N_USERS = 1_000_000
N_ITEMS = 100_000
N_M0 = 1_000
N_M1 = 20_000

f32 = mybir.dt.float32
i16 = mybir.dt.int16
bf16 = mybir.dt.bfloat16

# t-slots per chunk; 2 t-slots = 1024 gather indices = the max one
# dma_gather can carry (the SWDGE descriptor ring holds 1024 descriptors;
# num_idxs=2048 faults the exec unit, 1024 is verified good on HW).
CHUNKS = (2, 2, 2, 2, 2, 2, 2, 2)
ELEM = 128                 # bf16 elements per table row (= 256 B, required)
NLIN = 2                   # lin_hi, lin_lo columns
SHARD_N = (BL, BL, N_M0, N_M1)               # user, item, meta0, meta1
BASES = (0, BL, 2 * BL, 2 * BL + N_M0)
N_ROWS = 2 * BL + N_M0 + N_M1                # 25096 (< 2^15)
N_IDX = NF * BL                              # 8192 gathered rows per core


def build_nc():
    nc = bacc.Bacc(None, target_bir_lowering=False)
    idx_d = nc.declare_dram_parameter("idx", [P, N_IDX // 16], i16, isOutput=False)
    tab_d = nc.declare_dram_parameter("table", [N_ROWS, ELEM], bf16, isOutput=False)
    out_d = nc.declare_dram_parameter("out", [P, T], f32, isOutput=True)

    C = len(CHUNKS)
    with contextlib.ExitStack() as ctx:
        idx_sb = ctx.enter_context(nc.sbuf_tensor("idx_sb", [P, N_IDX // 16], i16))
        gs = [
            ctx.enter_context(nc.sbuf_tensor(f"g{k}", [P, NF * TC, ELEM], bf16))
            for k, TC in enumerate(CHUNKS)
        ]
        tmps = [
            [
                ctx.enter_context(nc.sbuf_tensor(f"tmp{k}_{j}", [P, TC, F], bf16))
                for j in range(7)
            ]
            for k, TC in enumerate(CHUNKS)
        ]
        pws = [ctx.enter_context(nc.sbuf_tensor(f"pw{k}", [P, TC], bf16))
               for k, TC in enumerate(CHUNKS)]
        ls = [ctx.enter_context(nc.sbuf_tensor(f"l{k}", [P, TC], f32))
              for k, TC in enumerate(CHUNKS)]
        zs = [ctx.enter_context(nc.sbuf_tensor(f"z{k}", [P, TC], f32))
              for k, TC in enumerate(CHUNKS)]
        sig = ctx.enter_context(nc.sbuf_tensor("sig", [P, T], f32))
        isem = ctx.enter_context(nc.semaphore("isem"))   # idx DMA, +16
        gsems = [ctx.enter_context(nc.semaphore(f"gsem{k}"))
                 for k in range(C)]                          # per-chunk gather DMA
        vsem = ctx.enter_context(nc.semaphore("vsem"))   # DVE chunk done, +1
        ssem = ctx.enter_context(nc.semaphore("ssem"))   # ACT chunk done, +1
        osem = ctx.enter_context(nc.semaphore("osem"))   # out DMAs, +16 each
        block = ctx.enter_context(nc.Block())

        @block.gpsimd
        def _(gpsimd):
            gpsimd.wait_ge(isem, 16)
            c0 = 0
            for k, TC in enumerate(CHUNKS):
                n_k = 128 * NF * TC
                gpsimd.dma_gather(
                    out_ap=gs[k][:],
                    in_ap=tab_d[:],
                    idxs_ap=idx_sb[:, c0:c0 + n_k // 16],
                    num_idxs=n_k,
                    num_idxs_reg=n_k,
                    elem_size=ELEM,
                ).then_inc(gsems[k], 16)
                c0 += n_k // 16

        @block.vector
        def _(vector):
            t0 = 0
            for k, TC in enumerate(CHUNKS):
                vector.wait_ge(gsems[k], 16)
                g4 = gs[k][:].rearrange("p (f t) e -> p f t e", f=NF)
                U = g4[:, 0, :, 0:F]
                I = g4[:, 1, :, 0:F]
                M0 = g4[:, 2, :, 0:F]
                M1 = g4[:, 3, :, 0:F]
                a, b, x, y, w, t2, d2 = tmps[k]
                vector.tensor_mul(out=x[:], in0=U, in1=I)
                vector.tensor_mul(out=y[:], in0=M0, in1=M1)
                vector.tensor_add(out=a[:], in0=U, in1=I)
                vector.tensor_add(out=b[:], in0=M0, in1=M1)
                vector.drain()          # DVE same-engine RAW needs a drain
                vector.tensor_mul(out=w[:], in0=a[:], in1=b[:])
                vector.tensor_add(out=t2[:], in0=x[:], in1=y[:])
                vector.drain()
                vector.tensor_add(out=d2[:], in0=t2[:], in1=w[:])
                vector.drain()
                with nc.allow_low_precision("FM pairwise term is O(0.05)"):
                    vector.tensor_reduce(
                        out=pws[k][:], in_=d2[:],
                        axis=mybir.AxisListType.X, op=mybir.AluOpType.add,
                    )
                lv = g4[:, :, :, F:F + NLIN].rearrange("p f t e -> p t f e")
                vector.tensor_reduce(
                    out=ls[k][:], in_=lv,
                    axis=mybir.AxisListType.XY, op=mybir.AluOpType.add,
                )
                vector.drain()
                vector.tensor_add(
                    out=zs[k][:], in0=pws[k][:], in1=ls[k][:]
                ).then_inc(vsem, 1)
                t0 += TC

        @block.scalar
        def _(scalar):
            t0 = 0
            for k, TC in enumerate(CHUNKS):
                scalar.wait_ge(vsem, k + 1)
                scalar.activation(
                    out=sig[:, t0:t0 + TC], in_=zs[k][:],
                    func=mybir.ActivationFunctionType.Sigmoid,
                ).then_inc(ssem, 1)
                t0 += TC
            scalar.drain()                   # sig writes land before HWDGE read
            scalar.dma_start(out=out_d[:], in_=sig[:]).then_inc(osem, 16)
            scalar.wait_ge(osem, 16)

        @block.sync
        def _(sync):
            sync.dma_start(out=idx_sb[:], in_=idx_d[:]).then_inc(isem, 16)

    nc.finalize()
    return nc


def _fill_rows(block, emb, lin):
    """block[:, :] = [emb | lin_hi | lin_lo | 0-pad] in bf16."""
    block[:, :F] = emb                          # f32 -> bf16 cast
    hi = lin.astype(ml_dtypes.bfloat16)
    block[:, F] = hi
    block[:, F + 1] = lin - hi.astype(np.float32)
    block[:, F + NLIN:] = 0


def host_prepare(inputs):
    """Row-shard the tables per core and build device index tensors."""
    user_emb = np.asarray(inputs["user_emb"], np.float32)
    item_emb = np.asarray(inputs["item_emb"], np.float32)
    user_lin = np.asarray(inputs["user_lin"], np.float32).reshape(-1)
    item_lin = np.asarray(inputs["item_lin"], np.float32).reshape(-1)

    uids = np.asarray(inputs["user_ids"]).astype(np.int64)
    iids = np.asarray(inputs["item_ids"]).astype(np.int64)
    meta = np.asarray(inputs["metadata_ids"]).astype(np.int64)

    # meta blocks are shared by all cores
    meta_block = np.zeros((N_M0 + N_M1, ELEM), ml_dtypes.bfloat16)
    _fill_rows(meta_block[:N_M0], np.asarray(inputs["meta_emb0"], np.float32),
               np.asarray(inputs["meta_lin0"], np.float32).reshape(-1))
    _fill_rows(meta_block[N_M0:], np.asarray(inputs["meta_emb1"], np.float32),
               np.asarray(inputs["meta_lin1"], np.float32).reshape(-1))

    per_core_idx, per_core_tab = [], []
    for c in range(N_CORES):
        sl = slice(c * BL, (c + 1) * BL)
        u_uniq, u_inv = np.unique(uids[sl], return_inverse=True)
        i_uniq, i_inv = np.unique(iids[sl], return_inverse=True)

        tab = np.zeros((N_ROWS, ELEM), ml_dtypes.bfloat16)
        _fill_rows(tab[:len(u_uniq)], user_emb[u_uniq], user_lin[u_uniq])
        _fill_rows(tab[BL:BL + len(i_uniq)], item_emb[i_uniq], item_lin[i_uniq])
        tab[2 * BL:] = meta_block

        # local row index per field, [NF, P, T] (batch item b = p*16 + t)
        loc = np.empty((NF, P, T), np.int16)
        loc[0] = (u_inv + BASES[0]).reshape(P, T)
        loc[1] = (i_inv + BASES[1]).reshape(P, T)
        loc[2] = (meta[sl, 0] + BASES[2]).reshape(P, T)
        loc[3] = (meta[sl, 1] + BASES[3]).reshape(P, T)

        # unwrapped gather order: index j -> out[p = j%128, col = j//128];
        # col = f*TC + tt within a chunk
        blocks = []
        t0 = 0
        for TC in CHUNKS:
            u_k = np.ascontiguousarray(
                loc[:, :, t0:t0 + TC].transpose(0, 2, 1)   # [f, tt, p]
            ).reshape(-1)                                   # j = (f*TC+tt)*128+p
            blocks.append(u_k.reshape(-1, 16).T)            # [16, n_k/16]
            t0 += TC
        idx16 = np.concatenate(blocks, axis=1)              # [16, N_IDX/16]
        per_core_idx.append(np.tile(idx16, (P // 16, 1)))   # replicate to 128
        per_core_tab.append(tab)
    return per_core_idx, per_core_tab


_NC_CACHE = None


def _get_nc():
    global _NC_CACHE
    if _NC_CACHE is None:
        _NC_CACHE = build_nc()
    return _NC_CACHE


def kernel(**inputs) -> np.ndarray:
    nc = _get_nc()
    per_core_idx, per_core_tab = host_prepare(inputs)
    in_maps = [
        {"idx": per_core_idx[c], "table": per_core_tab[c]}
        for c in range(N_CORES)
    ]
    res = run_bass_kernel_spmd(nc, in_maps, list(range(N_CORES)))
    return np.concatenate(
        [res.results[c]["out"].reshape(-1) for c in range(N_CORES)]
    ).astype(np.float32)
